# revision 39
# baseline (speedup 1.0000x reference)
"""Trainium2 Bass kernel for a segmented tensor-product contraction.

Computation (per batch row z, channel u, segments of width U=128):
  out[z, so, u] += c_p * x0[i0[z], s0_p, u] * prod_k x1[z, sk_p, u]
for 256 paths of degree 1..3 over S=16 segments.

Strategy:
  - Data-parallel over z across 8 NeuronCores (512 rows each).
  - On-chip layout: one big SBUF "arena" [u (128 partitions) x (pos, z)]
    holding x1 segments (DMA'd in first-use order), gathered x0 rows,
    shared product slots and path temporaries.  A single arena tile lets
    any two elementwise ops be fused into one DVE instruction via a
    3D access pattern with an arbitrary inter-block stride (the per-
    instruction overhead is ~140 cycles, so pairing saves ~144ns/op).
  - x0 row gather: host builds one-hot(i0); TensorEngine computes
    x0g[s] = x0[:, s]^T @ onehot (gather + transpose for free).
  - Factorization (globally optimized): suffix products sg(s0,s) =
    x0g[s0]*x1[s] and pairs pr(a,b) = x1[a]*x1[b]; each d2/d3 path is
    one tensor_tensor; squares go to the Scalar engine.
  - Output accumulation on TensorEngine: per-path coefficient-diagonal
    matmul into a per-segment PSUM bank (exact f32 adds).  Paths are
    emitted so output segments COMPLETE one at a time; each finished
    segment is copied out and DMA'd while later segments still compute.
  - PE warmup burst of tiny matmuls unthrottles the HAM clock gate
    before the gather/path matmuls start.
"""

import os
from collections import defaultdict

import numpy as np

U = 128
S = 16
NELEM = 64
Z = 4096
NCORES = 8
ZS = Z // NCORES  # 512 rows per core

LAST_EXEC_NS = None
LAST_RESULTS = None

F32 = "float32"


def _parse_paths(idxs, coeffs):
    paths = []  # (degree, x1segs_sorted, s0, so, coeff)
    for idx, cf in zip(idxs, coeffs):
        d = idx.shape[1] - 2
        for r, c in zip(idx, cf):
            r = [int(v) for v in r]
            paths.append((d, tuple(sorted(r[:d])), r[d], r[d + 1], float(c)))
    return paths


def _options(p):
    """Candidate (products, form) decompositions for a path.

    Each option: (frozenset of product keys, form)
    form = (in0_ref, in1_ref) with refs ('x1',s) ('x0g',s) ('sg',(s0,s))
    ('pair',(a,b)); d1 form = (('sg',(s0,s)), None).
    """
    d, segs, s0, so, c = p
    if d == 1:
        k = ("sg", (s0, segs[0]))
        return [(frozenset([k]), (k, None))]
    if d == 2:
        a, b = segs
        return [
            (frozenset([("sg", (s0, b))]), (("x1", a), ("sg", (s0, b)))),
            (frozenset([("sg", (s0, a))]), (("x1", b), ("sg", (s0, a)))),
            (frozenset([("pair", (a, b))]), (("pair", (a, b)), ("x0g", s0))),
        ]
    a, b, cc = segs
    return [
        (
            frozenset([("pair", (a, b)), ("sg", (s0, cc))]),
            (("pair", (a, b)), ("sg", (s0, cc))),
        ),
        (
            frozenset([("pair", (a, cc)), ("sg", (s0, b))]),
            (("pair", (a, cc)), ("sg", (s0, b))),
        ),
        (
            frozenset([("pair", (b, cc)), ("sg", (s0, a))]),
            (("pair", (b, cc)), ("sg", (s0, a))),
        ),
    ]


def _optimize_group(gpaths, n_restarts=24, n_sweeps=30):
    """Choose per-path decomposition minimizing total unique products
    (local search with deterministic randomized restarts)."""
    import random

    opts = [_options(p) for p in gpaths]

    def run(seed):
        rng = random.Random(seed)
        choices = (
            [0] * len(gpaths)
            if seed < 0
            else [rng.randrange(len(o)) for o in opts]
        )
        for _ in range(n_sweeps):
            counts = defaultdict(int)
            for i in range(len(gpaths)):
                for k in opts[i][choices[i]][0]:
                    counts[k] += 1
            changed = False
            order = list(range(len(gpaths)))
            if seed >= 0:
                rng.shuffle(order)
            for i in order:
                best, best_cost = choices[i], None
                for j, (prods, _) in enumerate(opts[i]):
                    cost = 0.0
                    for k in prods:
                        others = counts[k] - (
                            1 if k in opts[i][choices[i]][0] else 0
                        )
                        cost += 1.0 / (1 + others)
                    if best_cost is None or cost < best_cost - 1e-9:
                        best, best_cost = j, cost
                if best != choices[i]:
                    for k in opts[i][choices[i]][0]:
                        counts[k] -= 1
                    for k in opts[i][best][0]:
                        counts[k] += 1
                    choices[i] = best
                    changed = True
            if not changed:
                break
        products = set()
        for i in range(len(gpaths)):
            products |= opts[i][choices[i]][0]
        return products, choices

    best_products, best_choices = run(-1)
    for seed in range(n_restarts):
        products, choices = run(seed)
        if len(products) < len(best_products):
            best_products, best_choices = products, choices
    forms = [opts[i][best_choices[i]][1] for i in range(len(gpaths))]
    return best_products, forms


def _build_plan(idxs, coeffs):
    """Full schedule with so-serial emission.

    Events per group:
       ('build', [atom x1..3], eng?)   atom=(key, in0_ref, in1_ref)
       ('path', [pinfo x1..3])         pinfo=(d, r1, r2, coeff, so)
       ('so_done', so)
    Multi-atom events become one DVE instruction via a 3D access pattern;
    member positions must form an arithmetic sequence with |step| <= 63
    positions (ISA TENSOR3D limit) in out/in0/in1 simultaneously.
    """
    paths = _parse_paths(idxs, coeffs)
    products, forms = _optimize_group(paths)
    part = (list(range(8)), list(range(8, 16)))

    # d2-term reuse: a d3 path (a,b,c,s0) whose sub-pair+s0 matches a d2
    # path reuses that path's raw term t=x1a*x1b*x0g[s0]: the d2 path
    # becomes MM-direct from a product slot ('d2t') and the d3 path
    # multiplies by the remaining x1 segment, dropping its own products.
    gi_of_so = {}
    for gidx, sos in enumerate(part):
        for so in sos:
            gi_of_so[so] = gidx
    d2_by_sig = {}
    # net-zero in practice: the d3 paths' freed products stay shared by
    # other paths, so atoms don't drop while slots grow.  Disabled.
    if os.environ.get("KERNEL_D2T", "0") == "1":
        for i, p in enumerate(paths):
            if p[0] == 2:
                a, b = p[1]
                d2_by_sig.setdefault((a, b, p[2]), []).append(i)
    d2t_def = {}       # key -> (in0_ref, in1_ref) build recipe
    so_prereq = defaultdict(set)  # so -> set of so's that must come first
    for i, p in enumerate(paths):
        if p[0] != 3:
            continue
        a, b, cc = p[1]
        s0 = p[2]
        for (x, y, rem) in ((a, b, cc), (a, cc, b), (b, cc, a)):
            qs = d2_by_sig.get((x, y, s0), [])
            pick = None
            for q in qs:
                gq, gp = gi_of_so[paths[q][3]], gi_of_so[p[3]]
                if gq < gp or (gq == gp and q != i):
                    pick = (q, gq, gp)
                    break
            if pick is None:
                continue
            q, gq, gp = pick

            def reaches(a, b, seen=None):
                # True if a must come before b is violated... i.e. b -> a
                if seen is None:
                    seen = set()
                if a == b:
                    return True
                for nxt in so_prereq.get(a, ()):
                    if nxt not in seen:
                        seen.add(nxt)
                        if reaches(nxt, b, seen):
                            return True
                return False

            so_q, so_p = paths[q][3], p[3]
            if gq == gp and so_q != so_p and reaches(so_q, so_p):
                continue  # adding so_q before so_p would create a cycle
            key = ("d2t", (x, y, s0))
            if key not in d2t_def:
                d2t_def[key] = forms[q]
                forms[q] = (key, None)
            forms[i] = (key, ("x1", rem))
            if gq == gp and so_q != so_p:
                so_prereq[so_p].add(so_q)
            break

    use = [set(), set()]
    for p, form in zip(paths, forms):
        gi = 0 if p[3] in part[0] else 1
        for r in form:
            if r and r[0] in ("sg", "pair", "d2t"):
                use[gi].add(r)
    shared = use[0] & use[1]
    uniq = [use[0] - shared, use[1] - shared]

    so_paths = defaultdict(list)
    for i, p in enumerate(paths):
        so_paths[p[3]].append(i)

    def so_order_for(sos, built0):
        built = set(built0)
        remaining = [so for so in sos if so_paths[so]]
        placed = set(so for so in sos if not so_paths[so])
        order = []
        while remaining:
            best, bestc = None, None
            for so in remaining:
                if any(
                    pre in remaining for pre in so_prereq.get(so, ())
                ):
                    continue
                need = set()
                for i in so_paths[so]:
                    for r in forms[i]:
                        if (
                            r
                            and r[0] in ("sg", "pair", "d2t")
                            and r not in built
                        ):
                            need.add(r)
                c = (len(need), len(so_paths[so]))
                if bestc is None or c < bestc:
                    best, bestc = so, c
            order.append(best)
            remaining.remove(best)
            for i in so_paths[best]:
                for r in forms[i]:
                    if r and r[0] in ("sg", "pair", "d2t"):
                        built.add(r)
        # finish on a small segment so the final MM/copy/DMA tail is short
        if len(order) > 2:
            cands = [
                so
                for so in order[len(order) // 2 :]
                if not any(so in so_prereq.get(o, ()) for o in order)
            ]
            if cands:
                tail = min(cands, key=lambda so: len(so_paths[so]))
                order.remove(tail)
                order.append(tail)
        return order

    ns = len(shared)
    base_uniq = ns
    slot_of = {}       # shared products -> 0..ns-1 (assigned in emit order)
    next_shared = [0]
    uniq_slot = [{}, {}]
    next_uniq = [0, 0]

    schedules = []
    seg_first_use = []
    x0g_order = []

    def want_seg(s):
        if s not in seg_first_use:
            seg_first_use.append(s)
        return seg_first_use.index(s)

    def want_x0g(s0):
        if s0 not in x0g_order:
            x0g_order.append(s0)

    def ref_pos(r, gi):
        if r[0] == "x1":
            return want_seg(r[1])
        if r[0] == "x0g":
            want_x0g(r[1])
            return S + r[1]
        if r in slot_of:
            return 2 * S + slot_of[r]
        return 2 * S + base_uniq + uniq_slot[gi][r]

    def group_items(coords):
        """Greedy triples-then-pairs over (a,b) position pairs with swap
        freedom.  Returns list of (indices, swaps)."""
        n = len(coords)
        used = [False] * n
        groups = []
        ok = lambda d: abs(d) <= 63

        def tri_ok(x, y, z, sx, sy, sz):
            for c in (0, 1):
                ax = coords[x][c ^ sx]
                ay = coords[y][c ^ sy]
                az = coords[z][c ^ sz]
                if ay - ax != az - ay or not ok(ay - ax):
                    return False
            return True

        for i in range(n):
            if used[i]:
                continue
            found = None
            for j in range(i + 1, n):
                if used[j] or found:
                    continue
                for k in range(j + 1, n):
                    if used[k] or found:
                        continue
                    for order in ((i, j, k), (i, k, j), (j, i, k)):
                        if found:
                            break
                        for sw in range(8):
                            sx, sy, sz = sw & 1, (sw >> 1) & 1, (sw >> 2) & 1
                            if tri_ok(*order, sx, sy, sz):
                                found = (order, (sx, sy, sz))
                                break
            if found:
                order, sws = found
                for t in order:
                    used[t] = True
                groups.append((list(order), list(sws)))
        for i in range(n):
            if used[i]:
                continue
            used[i] = True
            found = None
            for j in range(i + 1, n):
                if used[j]:
                    continue
                (a0, a1), (b0, b1) = coords[i], coords[j]
                if ok(b0 - a0) and ok(b1 - a1):
                    found = (j, 0)
                    break
                if ok(b1 - a0) and ok(b0 - a1):
                    found = (j, 1)
                    break
            if found:
                j, sw = found
                used[j] = True
                groups.append(([i, j], [0, sw]))
            else:
                groups.append(([i], [0]))
        return groups

    for gi in (0, 1):
        built = set()
        if gi == 1:
            built |= shared  # shared products stay resident from group 0
        order = so_order_for(part[gi], built)
        sched = []

        for so in order:
            plist = sorted(
                so_paths[so], key=lambda i: forms[i][1] is not None
            )
            # outstanding build atoms, clustered (pair-kind first: no x0g
            # dependency; d2t last: it reads other products)
            clusters = {
                (k, reg)
                for k in ("pair", "sg", "d2t")
                for reg in (0, 1)
            }
            clusters = {kr: [] for kr in clusters}

            def add_product(r):
                if r in built:
                    return
                built.add(r)
                reg = 0 if r in shared else 1
                if r[0] == "sg":
                    clusters[("sg", reg)].append(
                        (r, ("x0g", r[1][0]), ("x1", r[1][1]))
                    )
                elif r[0] == "pair":
                    clusters[("pair", reg)].append(
                        (r, ("x1", r[1][0]), ("x1", r[1][1]))
                    )
                else:  # d2t: first its constituent products
                    ra, rb = d2t_def[r]
                    for rr in (ra, rb):
                        if rr and rr[0] in ("sg", "pair"):
                            add_product(rr)
                    clusters[("d2t", reg)].append((r, ra, rb))

            for i in plist:
                for r in forms[i]:
                    if not r or r[0] not in ("sg", "pair", "d2t"):
                        continue
                    add_product(r)
            def assign_slot(key, reg):
                if reg == 0:
                    slot_of[key] = next_shared[0]
                    next_shared[0] += 1
                else:
                    uniq_slot[gi][key] = next_uniq[gi]
                    next_uniq[gi] += 1

            def peek_slot_pos(reg):
                if reg == 0:
                    return 2 * S + next_shared[0]
                return 2 * S + base_uniq + next_uniq[gi]

            leftover = []
            for kind in ("pair", "sg", "d2t"):
                for reg in (0, 1):
                    atoms = clusters[(kind, reg)]
                    if not atoms:
                        continue
                    if kind == "pair":
                        atoms.sort(
                            key=lambda a: max(
                                ref_pos(a[1], gi), ref_pos(a[2], gi)
                            )
                        )
                    coords = [
                        (ref_pos(a[1], gi), ref_pos(a[2], gi)) for a in atoms
                    ]
                    for idx_list, sws in group_items(coords):
                        if len(idx_list) == 1:
                            leftover.append((atoms[idx_list[0]], reg))
                            continue
                        evatoms = []
                        for t, sw in zip(idx_list, sws):
                            key, ra, rb = atoms[t]
                            assign_slot(key, reg)
                            if sw:
                                ra, rb = rb, ra
                            evatoms.append((key, ra, rb))
                        sched.append(("build", evatoms))
            # merge cluster leftovers across kind/region where feasible
            lused = [False] * len(leftover)
            for i in range(len(leftover)):
                if lused[i]:
                    continue
                lused[i] = True
                (ki, ai, bi), ri = leftover[i]
                pi = (ref_pos(ai, gi), ref_pos(bi, gi))
                match = None
                for j in range(i + 1, len(leftover)):
                    if lused[j]:
                        continue
                    (kj, aj, bj), rj = leftover[j]
                    # candidate out positions
                    oi = peek_slot_pos(ri)
                    oj = peek_slot_pos(rj) + (1 if rj == ri else 0)
                    if abs(oj - oi) > 63:
                        continue
                    pj = (ref_pos(aj, gi), ref_pos(bj, gi))
                    if abs(pj[0] - pi[0]) <= 63 and abs(pj[1] - pi[1]) <= 63:
                        match = (j, 0)
                        break
                    if abs(pj[1] - pi[0]) <= 63 and abs(pj[0] - pi[1]) <= 63:
                        match = (j, 1)
                        break
                if match is None:
                    assign_slot(ki, ri)
                    sched.append(("build", [(ki, ai, bi)]))
                else:
                    j, sw = match
                    lused[j] = True
                    (kj, aj, bj), rj = leftover[j]
                    assign_slot(ki, ri)
                    assign_slot(kj, rj)
                    if sw:
                        aj, bj = bj, aj
                    sched.append(("build", [(ki, ai, bi), (kj, aj, bj)]))
            # paths: d1 singles first, then grouped d2/d3
            pend = []
            for i in plist:
                d, segs, s0, _, c = paths[i]
                r1, r2 = forms[i]
                if r2 is None:
                    sched.append(("path", [(d, r1, r2, c, so)]))
                else:
                    pend.append((d, r1, r2, c, so))
            coords = [(ref_pos(p[1], gi), ref_pos(p[2], gi)) for p in pend]
            for idx_list, sws in group_items(coords):
                ev = []
                for t, sw in zip(idx_list, sws):
                    d, r1, r2, c, soi = pend[t]
                    if sw:
                        r1, r2 = r2, r1
                    ev.append((d, r1, r2, c, soi))
                sched.append(("path", ev))
            sched.append(("so_done", so))
        schedules.append(sched)

    # merge leftover single-build events across neighboring so-sections
    # (the later section's build runs early, which is dependency-safe)
    def ref_pos_final(r, gi):
        if r[0] == "x1":
            return seg_first_use.index(r[1])
        if r[0] == "x0g":
            return S + r[1]
        if r in slot_of:
            return 2 * S + slot_of[r]
        return 2 * S + base_uniq + uniq_slot[gi][r]

    for gi, sched in enumerate(schedules):
        # section index of each event (so-clusters); skip the first two
        # sections of group 0 — their inputs are still arriving via DMA
        sec = 0
        sec_of = []
        for ev in sched:
            sec_of.append(sec)
            if ev[0] == "so_done":
                sec += 1
        singles = [
            (idx, ev)
            for idx, ev in enumerate(sched)
            if ev[0] == "build"
            and len(ev[1]) == 1
            and not (gi == 0 and sec_of[idx] < 2)
        ]
        used = set()
        for a in range(len(singles)):
            if a in used:
                continue
            ia, eva = singles[a]
            (ka, ra, rb) = eva[1][0]
            oa = 2 * S + (
                slot_of[ka] if ka in slot_of else base_uniq + uniq_slot[gi][ka]
            )
            pa = (ref_pos_final(ra, gi), ref_pos_final(rb, gi))
            for b in range(a + 1, len(singles)):
                if b in used:
                    continue
                ib, evb = singles[b]
                (kb, rc, rd) = evb[1][0]
                ob = 2 * S + (
                    slot_of[kb]
                    if kb in slot_of
                    else base_uniq + uniq_slot[gi][kb]
                )
                if abs(ob - oa) > 63:
                    continue
                pb = (ref_pos_final(rc, gi), ref_pos_final(rd, gi))
                sw = None
                if abs(pb[0] - pa[0]) <= 63 and abs(pb[1] - pa[1]) <= 63:
                    sw = 0
                elif abs(pb[1] - pa[0]) <= 63 and abs(pb[0] - pa[1]) <= 63:
                    sw = 1
                if sw is None:
                    continue
                used.add(a)
                used.add(b)
                atom_b = (kb, rd, rc) if sw else (kb, rc, rd)
                sched[ia] = ("build", [eva[1][0], atom_b])
                sched[ib] = None
                break
        schedules[gi] = [ev for ev in sched if ev is not None]

    # GPSIMD offload: hoist up to `gps_hoist` build instructions from each
    # so-section into the PREVIOUS section (same group), tagged to run on
    # GpSimd concurrently with the previous section's DVE work.
    gps_hoist = int(os.environ.get("KERNEL_GPS_HOIST", "0"))
    if gps_hoist > 0:
        new_schedules = []
        for sched in schedules:
            sections = []
            cur = []
            for ev in sched:
                cur.append(ev)
                if ev[0] == "so_done":
                    sections.append(cur)
                    cur = []
            if cur:
                sections.append(cur)
            for k in range(1, len(sections)):
                builds = [ev for ev in sections[k] if ev[0] == "build"]
                pick = builds[-gps_hoist:]
                hoisted = [("build", ev[1], "g") for ev in pick]
                rest = [ev for ev in sections[k] if ev not in pick]
                sections[k] = rest
                sections[k - 1] = hoisted + sections[k - 1]
            new_schedules.append([ev for sec in sections for ev in sec])
        schedules = new_schedules

    for s in range(S):
        want_seg(s)
    for s in range(S):
        want_x0g(s)

    assert next_shared[0] == ns, (next_shared, ns)
    n_slots = base_uniq + max(next_uniq[0], next_uniq[1])
    slot_maps = []
    for gi in (0, 1):
        m = dict(slot_of)
        for r, v in uniq_slot[gi].items():
            m[r] = base_uniq + v
        slot_maps.append(m)

    return dict(
        so_groups=part,
        schedules=schedules,
        slot_maps=slot_maps,
        n_slots=n_slots,
        seg_first_use=seg_first_use,
        x0g_order=x0g_order,
        paths=paths,
    )


N_TMP = 30  # rotating path-temporary slots in the arena
OUT_RING = 4  # output staging ring (segments)
X1_CHUNKS = (2, 2, 4, 4, 4)  # x1 DMA chunk sizes, first-use order
SLAB_SIZES = (8, 16) + (24,) * 12  # coeff-diagonal matrices per DMA slab


def _build_bass(plan, dtype_name, warmup):
    import concourse.bacc as bacc
    import concourse.mybir as mybir
    from concourse.ap import AP
    from concourse.tile import TileContext

    dt = mybir.dt.float32 if dtype_name == F32 else mybir.dt.bfloat16
    MULT = mybir.AluOpType.mult

    nc = bacc.Bacc("TRN2", debug=False)

    schedules = plan["schedules"]
    slot_maps = plan["slot_maps"]
    n_slots = plan["n_slots"]
    seg_first_use = plan["seg_first_use"]
    x0g_order = plan["x0g_order"]

    n_mms = sum(
        len(ev[1]) for sched in schedules for ev in sched if ev[0] == "path"
    )
    slab_sizes = []
    rem = n_mms
    for sz in SLAB_SIZES:
        if rem <= 0:
            break
        slab_sizes.append(min(sz, rem))
        rem -= slab_sizes[-1]
    while rem > 0:
        slab_sizes.append(min(24, rem))
        rem -= slab_sizes[-1]
    slab_off = [0]
    for sz in slab_sizes:
        slab_off.append(slab_off[-1] + sz)
    max_slab = max(slab_sizes)

    pos_of_seg = {s: i for i, s in enumerate(seg_first_use)}

    x1t_d = nc.dram_tensor("x1t", [S * U, ZS], dt, kind="ExternalInput")
    x0_d = nc.dram_tensor("x0w", [NELEM, S * U], dt, kind="ExternalInput")
    oh_d = nc.dram_tensor("oh", [NELEM, ZS], dt, kind="ExternalInput")
    cd_d = nc.dram_tensor("cdiag", [U, slab_off[-1] * U], dt, kind="ExternalInput")
    out_d = nc.dram_tensor("outt", [S * U, ZS], dt, kind="ExternalOutput")

    POS_X1 = 0
    POS_X0G = S
    POS_PROD = 2 * S
    POS_TMP = POS_PROD + n_slots
    N_ARENA = POS_TMP + N_TMP

    coeff_order = []

    with TileContext(nc) as tc:
        with tc.tile_pool(name="persist", bufs=1) as persist, tc.tile_pool(
            name="slab", bufs=2
        ) as slab_pool:
            arena = persist.tile([U, N_ARENA * ZS], dt, tag="arena")
            out_sb = persist.tile([U, OUT_RING * ZS], dt, tag="out")
            x0_sb = persist.tile([NELEM, S * U], dt, tag="x0w")
            oh_sb = persist.tile([NELEM, ZS], dt, tag="oh")
            wm_w = persist.tile([U, 32], dt, tag="wmw")
            wm_r = persist.tile([U, 256], dt, tag="wmr")

            pstep = [int(arena[:, 0:ZS].ap[0][0]), U]

            def seg(pos, n=1):
                return arena[:, pos * ZS : (pos + n) * ZS]

            def ref_pos(r, smap):
                kind, key = r
                if kind == "x1":
                    return POS_X1 + pos_of_seg[key]
                if kind == "x0g":
                    return POS_X0G + key
                return POS_PROD + smap[r]

            def group_ap(pos_list):
                k = len(pos_list)
                if k == 1:
                    return seg(pos_list[0])
                d = pos_list[1] - pos_list[0]
                assert abs(d) <= 63, pos_list
                if k == 3:
                    assert pos_list[2] - pos_list[1] == d, pos_list
                a = seg(pos_list[0])
                return AP(a.tensor, a.offset, [pstep, [d * ZS, k], [1, ZS]])

            if warmup > 0:
                nc.gpsimd.memset(wm_w[:], 0.0)
                nc.gpsimd.memset(wm_r[:], 0.0)

            # DMA layout: the scalar-triggered queue carries the small
            # gather inputs (done before the DVE stream starts) and later
            # the per-segment outputs; the sync queue carries x1 chunks
            # and the bulk coeff-diagonal slabs, serialized so only one
            # queue streams during the DVE-bound phase (concurrent bulk
            # DMA costs ~20% DVE throughput).
            nc.scalar.dma_start(out=x0_sb[:], in_=x0_d[:])
            nc.scalar.dma_start(out=oh_sb[:], in_=oh_d[:])
            ci = 0
            slabs_started = {}

            def start_slab(si, st):
                # plain 2D copy (host pre-transposed cdiag): 128 descriptors
                nc.sync.dma_start(
                    out=st[:, : slab_sizes[si] * U],
                    in_=cd_d[:, slab_off[si] * U : slab_off[si + 1] * U],
                )

            for ki, csz in enumerate(X1_CHUNKS):
                nc.sync.dma_start(
                    out=seg(POS_X1 + ci, csz).rearrange("p (g z) -> p g z", g=csz),
                    in_=x1t_d[ci * U : (ci + csz) * U, :].rearrange(
                        "(g p) z -> p g z", p=U
                    ),
                )
                ci += csz
                if ki == 2:
                    st = slab_pool.tile([U, max_slab * U], dt, tag="slab", name="slab0")
                    slabs_started[0] = st
                    start_slab(0, st)
                if ki == 3:
                    st = slab_pool.tile([U, max_slab * U], dt, tag="slab", name="slab1")
                    slabs_started[1] = st
                    start_slab(1, st)

            # PE warmup + gathers
            with tc.tile_pool(name="gpsum", bufs=4, space="PSUM") as gpsum:
                if warmup > 0:
                    wt = gpsum.tile([32, 256], mybir.dt.float32, tag="warm", bufs=1)
                    for i in range(warmup):
                        nc.tensor.matmul(
                            wt[:],
                            wm_w[:, 0:32],
                            wm_r[:],
                            start=(i == 0),
                            stop=(i == warmup - 1),
                        )
                    nc.scalar.copy(out=wm_r[:32, 0:8], in_=wt[:, 0:8])
                for s0 in x0g_order:
                    pt = gpsum.tile([U, ZS], mybir.dt.float32, tag="gps")
                    nc.tensor.matmul(
                        pt[:],
                        x0_sb[:, s0 * U : (s0 + 1) * U],
                        oh_sb[:],
                        start=True,
                        stop=True,
                    )
                    nc.scalar.copy(out=seg(POS_X0G + s0), in_=pt[:])

            tmp_next = [0]

            def alloc_tmp(n):
                base = tmp_next[0] % N_TMP
                if base + n > N_TMP:
                    tmp_next[0] += N_TMP - base
                    base = 0
                tmp_next[0] += n
                return POS_TMP + base

            slab_state = {"idx": -1, "tile": None}

            def weight_for(c):
                gidx = len(coeff_order)
                coeff_order.append(c)
                si = 0
                while slab_off[si + 1] <= gidx:
                    si += 1
                if slab_state["idx"] != si:
                    slab_state["idx"] = si
                    if si in slabs_started:
                        st = slabs_started[si]
                    else:
                        st = slab_pool.tile(
                            [U, max_slab * U], dt, tag="slab", name=f"slab{si}"
                        )
                        start_slab(si, st)
                    slab_state["tile"] = st
                st = slab_state["tile"]
                sk = gidx - slab_off[si]
                return st[:, sk * U : (sk + 1) * U]

            so_count = [0]

            for gi, sched in enumerate(schedules):
                smap = slot_maps[gi]
                sos = plan["so_groups"][gi]
                mm_idx = 0
                first_mm = {}
                last_mm = {}
                for ev in sched:
                    if ev[0] != "path":
                        continue
                    for d, r1, r2, c, so in ev[1]:
                        if so not in first_mm:
                            first_mm[so] = mm_idx
                        last_mm[so] = mm_idx
                        mm_idx += 1

                acc = {}
                with tc.tile_pool(
                    name=f"acc{gi}", bufs=8, space="PSUM"
                ) as acc_pool:
                    mm_idx = 0
                    for ev in sched:
                        kind = ev[0]
                        if kind == "build":
                            atoms = ev[1]
                            eng = nc.gpsimd if len(ev) > 2 and ev[2] == "g" else nc.vector
                            outs = [POS_PROD + smap[a[0]] for a in atoms]
                            i0s = [ref_pos(a[1], smap) for a in atoms]
                            i1s = [ref_pos(a[2], smap) for a in atoms]
                            eng.tensor_tensor(
                                out=group_ap(outs),
                                in0=group_ap(i0s),
                                in1=group_ap(i1s),
                                op=MULT,
                            )
                        elif kind == "path":
                            pinfos = ev[1]
                            if pinfos[0][2] is not None:
                                k = len(pinfos)
                                t0 = alloc_tmp(k)
                                nc.vector.tensor_tensor(
                                    out=group_ap(list(range(t0, t0 + k))),
                                    in0=group_ap(
                                        [ref_pos(p[1], smap) for p in pinfos]
                                    ),
                                    in1=group_ap(
                                        [ref_pos(p[2], smap) for p in pinfos]
                                    ),
                                    op=MULT,
                                )
                                rhs_pos = list(range(t0, t0 + k))
                            else:
                                rhs_pos = [ref_pos(pinfos[0][1], smap)]
                            for (d, r1, r2, c, so), rp in zip(pinfos, rhs_pos):
                                if so not in acc:
                                    acc[so] = acc_pool.tile(
                                        [U, ZS],
                                        mybir.dt.float32,
                                        tag=f"acc{sos.index(so)}",
                                        name=f"acc_{so}",
                                        bufs=1,
                                    )
                                blk = weight_for(c)
                                nc.tensor.matmul(
                                    acc[so][:],
                                    blk,
                                    seg(rp),
                                    start=(mm_idx == first_mm[so]),
                                    stop=(mm_idx == last_mm[so]),
                                )
                                mm_idx += 1
                        else:  # so_done
                            so = ev[1]
                            ring = so_count[0] % OUT_RING
                            so_count[0] += 1
                            if so in acc:
                                nc.scalar.copy(
                                    out=out_sb[:, ring * ZS : (ring + 1) * ZS],
                                    in_=acc[so][:],
                                )
                            else:
                                nc.vector.memset(
                                    out_sb[:, ring * ZS : (ring + 1) * ZS], 0.0
                                )
                            nc.scalar.dma_start(
                                out=out_d[so * U : (so + 1) * U, :],
                                in_=out_sb[:, ring * ZS : (ring + 1) * ZS],
                            )
                    done = set(
                        ev[1] for ev in sched if ev[0] == "so_done"
                    )
                    for so in sos:
                        if so not in done:
                            ring = so_count[0] % OUT_RING
                            so_count[0] += 1
                            nc.vector.memset(
                                out_sb[:, ring * ZS : (ring + 1) * ZS], 0.0
                            )
                            nc.scalar.dma_start(
                                out=out_d[so * U : (so + 1) * U, :],
                                in_=out_sb[:, ring * ZS : (ring + 1) * ZS],
                            )

    nc.compile()
    return nc, coeff_order


def _ensure_ntff_hook():
    """Register the axon NTFF profiling hook if the image's antenv lacks
    axon_hooks (bass_utils needs it for trace=True under axon)."""
    import sys
    import types

    try:
        import antenv.axon_hooks  # noqa: F401

        return
    except ImportError:
        pass
    import contextlib
    import ctypes

    so_path = "/opt/axon/libaxon_pjrt.so"
    if not os.path.exists(so_path):
        return
    lib = ctypes.CDLL(so_path)
    if not hasattr(lib, "axon_start_nrt_profile"):
        return
    lib.axon_start_nrt_profile.argtypes = [
        ctypes.POINTER(ctypes.c_int64),
        ctypes.c_size_t,
    ]
    lib.axon_start_nrt_profile.restype = ctypes.c_int64
    lib.axon_stop_nrt_profile.argtypes = [ctypes.c_char_p]
    lib.axon_stop_nrt_profile.restype = ctypes.c_int64

    @contextlib.contextmanager
    def _hook(output_dir, device_ids):
        import jax

        jax.devices()
        if device_ids:
            ids = (ctypes.c_int64 * len(device_ids))(*device_ids)
            rc = lib.axon_start_nrt_profile(ids, len(device_ids))
        else:
            rc = lib.axon_start_nrt_profile(None, 0)
        if rc != 0:
            raise RuntimeError(f"axon_start_nrt_profile rc={rc}")
        try:
            yield
        finally:
            n = lib.axon_stop_nrt_profile(str(output_dir).encode())
            print(f"profile: {n} file(s) written to {output_dir}")

    mod = types.ModuleType("antenv.axon_hooks")
    state = {"hook": _hook}
    mod.get_axon_ntff_profile_hook = lambda: state["hook"]
    mod.set_axon_ntff_profile_hook = lambda h: state.__setitem__("hook", h)
    import antenv

    antenv.axon_hooks = mod
    sys.modules["antenv.axon_hooks"] = mod


def kernel(x0, x1, coeff1, coeff2, coeff3, i0, idx1, idx2, idx3):
    global LAST_EXEC_NS, LAST_RESULTS
    from concourse.bass_utils import run_bass_kernel_spmd

    x0 = np.asarray(x0, dtype=np.float32)
    x1 = np.asarray(x1, dtype=np.float32)
    i0 = np.asarray(i0).astype(np.int64)
    idxs = [np.asarray(a) for a in (idx1, idx2, idx3)]
    coeffs = [np.asarray(c, dtype=np.float32) for c in (coeff1, coeff2, coeff3)]

    dtype_name = os.environ.get("KERNEL_DTYPE", "bfloat16")
    warmup = int(os.environ.get("KERNEL_WARMUP", "12"))
    npdt = np.float32
    if dtype_name != F32:
        import ml_dtypes

        npdt = ml_dtypes.bfloat16

    plan = _build_plan(idxs, coeffs)
    nc, coeff_order = _build_bass(plan, dtype_name, warmup)

    # replicate the device-side slab trimming exactly
    n_co = len(coeff_order)
    total = 0
    rem = n_co
    for sz in SLAB_SIZES:
        if rem <= 0:
            break
        take = min(sz, rem)
        total += take
        rem -= take
    while rem > 0:
        take = min(24, rem)
        total += take
        rem -= take
    cdiag = np.zeros((U, total * U), dtype=npdt)
    uu = np.arange(U)
    for gidx, c in enumerate(coeff_order):
        cdiag[uu, gidx * U + uu] = npdt(c)

    # host-side layout: x1 transposed per core with segments permuted into
    # first-use order
    perm = plan["seg_first_use"]
    in_maps = []
    eye = np.arange(NELEM)
    x0c = x0.astype(npdt)
    for c in range(NCORES):
        zl, zh = c * ZS, (c + 1) * ZS
        shard = x1[zl:zh].reshape(ZS, S, U)
        x1t = np.ascontiguousarray(
            shard[:, perm, :].transpose(1, 2, 0).reshape(S * U, ZS)
        ).astype(npdt)
        oh = (i0[zl:zh][None, :] == eye[:, None]).astype(npdt)
        in_maps.append({"x1t": x1t, "x0w": x0c, "oh": oh, "cdiag": cdiag})

    trace = os.environ.get("BASS_TRACE", "") not in ("", "0")
    if trace:
        _ensure_ntff_hook()
    trace_cores = None
    tc_env = os.environ.get("KERNEL_TRACE_CORES", "")
    if tc_env:
        trace_cores = [int(x) for x in tc_env.split(",")]
    res = run_bass_kernel_spmd(
        nc, in_maps, core_ids=list(range(NCORES)), trace=trace,
        trace_cores=trace_cores,
    )
    LAST_EXEC_NS = res.exec_time_ns
    LAST_RESULTS = res

    out = np.empty((Z, S * U), dtype=np.float32)
    for c in range(NCORES):
        outt = np.asarray(res.results[c]["outt"], dtype=np.float32)
        out[c * ZS : (c + 1) * ZS] = (
            outt.reshape(S, U, ZS).transpose(2, 0, 1).reshape(ZS, S * U)
        )
    return out


# revision 40
# speedup vs baseline: 1.0039x; 1.0039x over previous
"""Trainium2 Bass kernel for a segmented tensor-product contraction.

Computation (per batch row z, channel u, segments of width U=128):
  out[z, so, u] += c_p * x0[i0[z], s0_p, u] * prod_k x1[z, sk_p, u]
for 256 paths of degree 1..3 over S=16 segments.

Strategy:
  - Data-parallel over z across 8 NeuronCores (512 rows each).
  - On-chip layout: one big SBUF "arena" [u (128 partitions) x (pos, z)]
    holding x1 segments (DMA'd in first-use order), gathered x0 rows,
    shared product slots and path temporaries.  A single arena tile lets
    any two elementwise ops be fused into one DVE instruction via a
    3D access pattern with an arbitrary inter-block stride (the per-
    instruction overhead is ~140 cycles, so pairing saves ~144ns/op).
  - x0 row gather: host builds one-hot(i0); TensorEngine computes
    x0g[s] = x0[:, s]^T @ onehot (gather + transpose for free).
  - Factorization (globally optimized): suffix products sg(s0,s) =
    x0g[s0]*x1[s] and pairs pr(a,b) = x1[a]*x1[b]; each d2/d3 path is
    one tensor_tensor; squares go to the Scalar engine.
  - Output accumulation on TensorEngine: per-path coefficient-diagonal
    matmul into a per-segment PSUM bank (exact f32 adds).  Paths are
    emitted so output segments COMPLETE one at a time; each finished
    segment is copied out and DMA'd while later segments still compute.
  - PE warmup burst of tiny matmuls unthrottles the HAM clock gate
    before the gather/path matmuls start.
"""

import os
from collections import defaultdict

import numpy as np

U = 128
S = 16
NELEM = 64
Z = 4096
NCORES = 8
ZS = Z // NCORES  # 512 rows per core

LAST_EXEC_NS = None
LAST_RESULTS = None

F32 = "float32"


def _parse_paths(idxs, coeffs):
    paths = []  # (degree, x1segs_sorted, s0, so, coeff)
    for idx, cf in zip(idxs, coeffs):
        d = idx.shape[1] - 2
        for r, c in zip(idx, cf):
            r = [int(v) for v in r]
            paths.append((d, tuple(sorted(r[:d])), r[d], r[d + 1], float(c)))
    return paths


def _options(p):
    """Candidate (products, form) decompositions for a path.

    Each option: (frozenset of product keys, form)
    form = (in0_ref, in1_ref) with refs ('x1',s) ('x0g',s) ('sg',(s0,s))
    ('pair',(a,b)); d1 form = (('sg',(s0,s)), None).
    """
    d, segs, s0, so, c = p
    if d == 1:
        k = ("sg", (s0, segs[0]))
        return [(frozenset([k]), (k, None))]
    if d == 2:
        a, b = segs
        return [
            (frozenset([("sg", (s0, b))]), (("x1", a), ("sg", (s0, b)))),
            (frozenset([("sg", (s0, a))]), (("x1", b), ("sg", (s0, a)))),
            (frozenset([("pair", (a, b))]), (("pair", (a, b)), ("x0g", s0))),
        ]
    a, b, cc = segs
    return [
        (
            frozenset([("pair", (a, b)), ("sg", (s0, cc))]),
            (("pair", (a, b)), ("sg", (s0, cc))),
        ),
        (
            frozenset([("pair", (a, cc)), ("sg", (s0, b))]),
            (("pair", (a, cc)), ("sg", (s0, b))),
        ),
        (
            frozenset([("pair", (b, cc)), ("sg", (s0, a))]),
            (("pair", (b, cc)), ("sg", (s0, a))),
        ),
    ]


def _optimize_group(gpaths, n_restarts=24, n_sweeps=30):
    """Choose per-path decomposition minimizing total unique products
    (local search with deterministic randomized restarts)."""
    import random

    opts = [_options(p) for p in gpaths]

    def run(seed):
        rng = random.Random(seed)
        choices = (
            [0] * len(gpaths)
            if seed < 0
            else [rng.randrange(len(o)) for o in opts]
        )
        for _ in range(n_sweeps):
            counts = defaultdict(int)
            for i in range(len(gpaths)):
                for k in opts[i][choices[i]][0]:
                    counts[k] += 1
            changed = False
            order = list(range(len(gpaths)))
            if seed >= 0:
                rng.shuffle(order)
            for i in order:
                best, best_cost = choices[i], None
                for j, (prods, _) in enumerate(opts[i]):
                    cost = 0.0
                    for k in prods:
                        others = counts[k] - (
                            1 if k in opts[i][choices[i]][0] else 0
                        )
                        cost += 1.0 / (1 + others)
                    if best_cost is None or cost < best_cost - 1e-9:
                        best, best_cost = j, cost
                if best != choices[i]:
                    for k in opts[i][choices[i]][0]:
                        counts[k] -= 1
                    for k in opts[i][best][0]:
                        counts[k] += 1
                    choices[i] = best
                    changed = True
            if not changed:
                break
        products = set()
        for i in range(len(gpaths)):
            products |= opts[i][choices[i]][0]
        return products, choices

    best_products, best_choices = run(-1)
    for seed in range(n_restarts):
        products, choices = run(seed)
        if len(products) < len(best_products):
            best_products, best_choices = products, choices
    forms = [opts[i][best_choices[i]][1] for i in range(len(gpaths))]
    return best_products, forms


def _build_plan(idxs, coeffs):
    """Full schedule with so-serial emission.

    Events per group:
       ('build', [atom x1..3], eng?)   atom=(key, in0_ref, in1_ref)
       ('path', [pinfo x1..3])         pinfo=(d, r1, r2, coeff, so)
       ('so_done', so)
    Multi-atom events become one DVE instruction via a 3D access pattern;
    member positions must form an arithmetic sequence with |step| <= 63
    positions (ISA TENSOR3D limit) in out/in0/in1 simultaneously.
    """
    paths = _parse_paths(idxs, coeffs)
    products, forms = _optimize_group(paths)
    part = (list(range(8)), list(range(8, 16)))

    # d2-term reuse: a d3 path (a,b,c,s0) whose sub-pair+s0 matches a d2
    # path reuses that path's raw term t=x1a*x1b*x0g[s0]: the d2 path
    # becomes MM-direct from a product slot ('d2t') and the d3 path
    # multiplies by the remaining x1 segment, dropping its own products.
    gi_of_so = {}
    for gidx, sos in enumerate(part):
        for so in sos:
            gi_of_so[so] = gidx
    d2_by_sig = {}
    # net-zero in practice: the d3 paths' freed products stay shared by
    # other paths, so atoms don't drop while slots grow.  Disabled.
    if os.environ.get("KERNEL_D2T", "0") == "1":
        for i, p in enumerate(paths):
            if p[0] == 2:
                a, b = p[1]
                d2_by_sig.setdefault((a, b, p[2]), []).append(i)
    d2t_def = {}       # key -> (in0_ref, in1_ref) build recipe
    so_prereq = defaultdict(set)  # so -> set of so's that must come first
    for i, p in enumerate(paths):
        if p[0] != 3:
            continue
        a, b, cc = p[1]
        s0 = p[2]
        for (x, y, rem) in ((a, b, cc), (a, cc, b), (b, cc, a)):
            qs = d2_by_sig.get((x, y, s0), [])
            pick = None
            for q in qs:
                gq, gp = gi_of_so[paths[q][3]], gi_of_so[p[3]]
                if gq < gp or (gq == gp and q != i):
                    pick = (q, gq, gp)
                    break
            if pick is None:
                continue
            q, gq, gp = pick

            def reaches(a, b, seen=None):
                # True if a must come before b is violated... i.e. b -> a
                if seen is None:
                    seen = set()
                if a == b:
                    return True
                for nxt in so_prereq.get(a, ()):
                    if nxt not in seen:
                        seen.add(nxt)
                        if reaches(nxt, b, seen):
                            return True
                return False

            so_q, so_p = paths[q][3], p[3]
            if gq == gp and so_q != so_p and reaches(so_q, so_p):
                continue  # adding so_q before so_p would create a cycle
            key = ("d2t", (x, y, s0))
            if key not in d2t_def:
                d2t_def[key] = forms[q]
                forms[q] = (key, None)
            forms[i] = (key, ("x1", rem))
            if gq == gp and so_q != so_p:
                so_prereq[so_p].add(so_q)
            break

    use = [set(), set()]
    for p, form in zip(paths, forms):
        gi = 0 if p[3] in part[0] else 1
        for r in form:
            if r and r[0] in ("sg", "pair", "d2t"):
                use[gi].add(r)
    shared = use[0] & use[1]
    uniq = [use[0] - shared, use[1] - shared]

    so_paths = defaultdict(list)
    for i, p in enumerate(paths):
        so_paths[p[3]].append(i)

    def so_order_for(sos, built0):
        built = set(built0)
        remaining = [so for so in sos if so_paths[so]]
        placed = set(so for so in sos if not so_paths[so])
        order = []
        while remaining:
            best, bestc = None, None
            for so in remaining:
                if any(
                    pre in remaining for pre in so_prereq.get(so, ())
                ):
                    continue
                need = set()
                for i in so_paths[so]:
                    for r in forms[i]:
                        if (
                            r
                            and r[0] in ("sg", "pair", "d2t")
                            and r not in built
                        ):
                            need.add(r)
                c = (len(need), len(so_paths[so]))
                if bestc is None or c < bestc:
                    best, bestc = so, c
            order.append(best)
            remaining.remove(best)
            for i in so_paths[best]:
                for r in forms[i]:
                    if r and r[0] in ("sg", "pair", "d2t"):
                        built.add(r)
        # finish on a small segment so the final MM/copy/DMA tail is short
        if len(order) > 2:
            cands = [
                so
                for so in order[len(order) // 2 :]
                if not any(so in so_prereq.get(o, ()) for o in order)
            ]
            if cands:
                tail = min(cands, key=lambda so: len(so_paths[so]))
                order.remove(tail)
                order.append(tail)
        return order

    ns = len(shared)
    base_uniq = ns
    slot_of = {}       # shared products -> 0..ns-1 (assigned in emit order)
    next_shared = [0]
    uniq_slot = [{}, {}]
    next_uniq = [0, 0]

    schedules = []
    seg_first_use = []
    x0g_order = []

    def want_seg(s):
        if s not in seg_first_use:
            seg_first_use.append(s)
        return seg_first_use.index(s)

    def want_x0g(s0):
        if s0 not in x0g_order:
            x0g_order.append(s0)

    def ref_pos(r, gi):
        if r[0] == "x1":
            return want_seg(r[1])
        if r[0] == "x0g":
            want_x0g(r[1])
            return S + r[1]
        if r in slot_of:
            return 2 * S + slot_of[r]
        return 2 * S + base_uniq + uniq_slot[gi][r]

    def group_items(coords):
        """Greedy triples-then-pairs over (a,b) position pairs with swap
        freedom.  Returns list of (indices, swaps)."""
        n = len(coords)
        used = [False] * n
        groups = []
        ok = lambda d: abs(d) <= 63

        def tri_ok(x, y, z, sx, sy, sz):
            for c in (0, 1):
                ax = coords[x][c ^ sx]
                ay = coords[y][c ^ sy]
                az = coords[z][c ^ sz]
                if ay - ax != az - ay or not ok(ay - ax):
                    return False
            return True

        for i in range(n):
            if used[i]:
                continue
            found = None
            for j in range(i + 1, n):
                if used[j] or found:
                    continue
                for k in range(j + 1, n):
                    if used[k] or found:
                        continue
                    for order in ((i, j, k), (i, k, j), (j, i, k)):
                        if found:
                            break
                        for sw in range(8):
                            sx, sy, sz = sw & 1, (sw >> 1) & 1, (sw >> 2) & 1
                            if tri_ok(*order, sx, sy, sz):
                                found = (order, (sx, sy, sz))
                                break
            if found:
                order, sws = found
                for t in order:
                    used[t] = True
                groups.append((list(order), list(sws)))
        for i in range(n):
            if used[i]:
                continue
            used[i] = True
            found = None
            for j in range(i + 1, n):
                if used[j]:
                    continue
                (a0, a1), (b0, b1) = coords[i], coords[j]
                if ok(b0 - a0) and ok(b1 - a1):
                    found = (j, 0)
                    break
                if ok(b1 - a0) and ok(b0 - a1):
                    found = (j, 1)
                    break
            if found:
                j, sw = found
                used[j] = True
                groups.append(([i, j], [0, sw]))
            else:
                groups.append(([i], [0]))
        return groups

    for gi in (0, 1):
        built = set()
        if gi == 1:
            built |= shared  # shared products stay resident from group 0
        order = so_order_for(part[gi], built)
        sched = []

        for so in order:
            plist = sorted(
                so_paths[so], key=lambda i: forms[i][1] is not None
            )
            # outstanding build atoms, clustered (pair-kind first: no x0g
            # dependency; d2t last: it reads other products)
            clusters = {
                (k, reg)
                for k in ("pair", "sg", "d2t")
                for reg in (0, 1)
            }
            clusters = {kr: [] for kr in clusters}

            def add_product(r):
                if r in built:
                    return
                built.add(r)
                reg = 0 if r in shared else 1
                if r[0] == "sg":
                    clusters[("sg", reg)].append(
                        (r, ("x0g", r[1][0]), ("x1", r[1][1]))
                    )
                elif r[0] == "pair":
                    clusters[("pair", reg)].append(
                        (r, ("x1", r[1][0]), ("x1", r[1][1]))
                    )
                else:  # d2t: first its constituent products
                    ra, rb = d2t_def[r]
                    for rr in (ra, rb):
                        if rr and rr[0] in ("sg", "pair"):
                            add_product(rr)
                    clusters[("d2t", reg)].append((r, ra, rb))

            for i in plist:
                for r in forms[i]:
                    if not r or r[0] not in ("sg", "pair", "d2t"):
                        continue
                    add_product(r)
            def assign_slot(key, reg):
                if reg == 0:
                    slot_of[key] = next_shared[0]
                    next_shared[0] += 1
                else:
                    uniq_slot[gi][key] = next_uniq[gi]
                    next_uniq[gi] += 1

            def peek_slot_pos(reg):
                if reg == 0:
                    return 2 * S + next_shared[0]
                return 2 * S + base_uniq + next_uniq[gi]

            leftover = []
            # group 0, first section: lead with a single square build (one
            # x1 segment) so the DVE starts as soon as the first 128KB
            # chunk lands
            if gi == 0 and not sched:
                for reg in (0, 1):
                    cl = clusters[("pair", reg)]
                    sq = next(
                        (a for a in cl if a[1] == a[2]), None
                    )
                    if sq is not None:
                        cl.remove(sq)
                        ref_pos(sq[1], gi)
                        assign_slot_first = sq[0]
                        if reg == 0:
                            slot_of[sq[0]] = next_shared[0]
                            next_shared[0] += 1
                        else:
                            uniq_slot[gi][sq[0]] = next_uniq[gi]
                            next_uniq[gi] += 1
                        sched.append(("build", [sq]))
                        break
            for kind in ("pair", "sg", "d2t"):
                for reg in (0, 1):
                    atoms = clusters[(kind, reg)]
                    if not atoms:
                        continue
                    if kind == "pair":
                        atoms.sort(
                            key=lambda a: max(
                                ref_pos(a[1], gi), ref_pos(a[2], gi)
                            )
                        )
                    coords = [
                        (ref_pos(a[1], gi), ref_pos(a[2], gi)) for a in atoms
                    ]
                    for idx_list, sws in group_items(coords):
                        if len(idx_list) == 1:
                            leftover.append((atoms[idx_list[0]], reg))
                            continue
                        evatoms = []
                        for t, sw in zip(idx_list, sws):
                            key, ra, rb = atoms[t]
                            assign_slot(key, reg)
                            if sw:
                                ra, rb = rb, ra
                            evatoms.append((key, ra, rb))
                        sched.append(("build", evatoms))
            # merge cluster leftovers across kind/region where feasible
            lused = [False] * len(leftover)
            for i in range(len(leftover)):
                if lused[i]:
                    continue
                lused[i] = True
                (ki, ai, bi), ri = leftover[i]
                pi = (ref_pos(ai, gi), ref_pos(bi, gi))
                match = None
                for j in range(i + 1, len(leftover)):
                    if lused[j]:
                        continue
                    (kj, aj, bj), rj = leftover[j]
                    # candidate out positions
                    oi = peek_slot_pos(ri)
                    oj = peek_slot_pos(rj) + (1 if rj == ri else 0)
                    if abs(oj - oi) > 63:
                        continue
                    pj = (ref_pos(aj, gi), ref_pos(bj, gi))
                    if abs(pj[0] - pi[0]) <= 63 and abs(pj[1] - pi[1]) <= 63:
                        match = (j, 0)
                        break
                    if abs(pj[1] - pi[0]) <= 63 and abs(pj[0] - pi[1]) <= 63:
                        match = (j, 1)
                        break
                if match is None:
                    assign_slot(ki, ri)
                    sched.append(("build", [(ki, ai, bi)]))
                else:
                    j, sw = match
                    lused[j] = True
                    (kj, aj, bj), rj = leftover[j]
                    assign_slot(ki, ri)
                    assign_slot(kj, rj)
                    if sw:
                        aj, bj = bj, aj
                    sched.append(("build", [(ki, ai, bi), (kj, aj, bj)]))
            # paths: d1 singles first, then grouped d2/d3
            pend = []
            for i in plist:
                d, segs, s0, _, c = paths[i]
                r1, r2 = forms[i]
                if r2 is None:
                    sched.append(("path", [(d, r1, r2, c, so)]))
                else:
                    pend.append((d, r1, r2, c, so))
            coords = [(ref_pos(p[1], gi), ref_pos(p[2], gi)) for p in pend]
            for idx_list, sws in group_items(coords):
                ev = []
                for t, sw in zip(idx_list, sws):
                    d, r1, r2, c, soi = pend[t]
                    if sw:
                        r1, r2 = r2, r1
                    ev.append((d, r1, r2, c, soi))
                sched.append(("path", ev))
            sched.append(("so_done", so))
        schedules.append(sched)

    # merge leftover single-build events across neighboring so-sections
    # (the later section's build runs early, which is dependency-safe)
    def ref_pos_final(r, gi):
        if r[0] == "x1":
            return seg_first_use.index(r[1])
        if r[0] == "x0g":
            return S + r[1]
        if r in slot_of:
            return 2 * S + slot_of[r]
        return 2 * S + base_uniq + uniq_slot[gi][r]

    for gi, sched in enumerate(schedules):
        # section index of each event (so-clusters); skip the first two
        # sections of group 0 — their inputs are still arriving via DMA
        sec = 0
        sec_of = []
        for ev in sched:
            sec_of.append(sec)
            if ev[0] == "so_done":
                sec += 1
        singles = [
            (idx, ev)
            for idx, ev in enumerate(sched)
            if ev[0] == "build"
            and len(ev[1]) == 1
            and not (gi == 0 and sec_of[idx] < 2)
        ]
        used = set()
        for a in range(len(singles)):
            if a in used:
                continue
            ia, eva = singles[a]
            (ka, ra, rb) = eva[1][0]
            oa = 2 * S + (
                slot_of[ka] if ka in slot_of else base_uniq + uniq_slot[gi][ka]
            )
            pa = (ref_pos_final(ra, gi), ref_pos_final(rb, gi))
            for b in range(a + 1, len(singles)):
                if b in used:
                    continue
                ib, evb = singles[b]
                (kb, rc, rd) = evb[1][0]
                ob = 2 * S + (
                    slot_of[kb]
                    if kb in slot_of
                    else base_uniq + uniq_slot[gi][kb]
                )
                if abs(ob - oa) > 63:
                    continue
                pb = (ref_pos_final(rc, gi), ref_pos_final(rd, gi))
                sw = None
                if abs(pb[0] - pa[0]) <= 63 and abs(pb[1] - pa[1]) <= 63:
                    sw = 0
                elif abs(pb[1] - pa[0]) <= 63 and abs(pb[0] - pa[1]) <= 63:
                    sw = 1
                if sw is None:
                    continue
                used.add(a)
                used.add(b)
                atom_b = (kb, rd, rc) if sw else (kb, rc, rd)
                sched[ia] = ("build", [eva[1][0], atom_b])
                sched[ib] = None
                break
        schedules[gi] = [ev for ev in sched if ev is not None]

    # GPSIMD offload: hoist up to `gps_hoist` build instructions from each
    # so-section into the PREVIOUS section (same group), tagged to run on
    # GpSimd concurrently with the previous section's DVE work.
    gps_hoist = int(os.environ.get("KERNEL_GPS_HOIST", "0"))
    if gps_hoist > 0:
        new_schedules = []
        for sched in schedules:
            sections = []
            cur = []
            for ev in sched:
                cur.append(ev)
                if ev[0] == "so_done":
                    sections.append(cur)
                    cur = []
            if cur:
                sections.append(cur)
            for k in range(1, len(sections)):
                builds = [ev for ev in sections[k] if ev[0] == "build"]
                pick = builds[-gps_hoist:]
                hoisted = [("build", ev[1], "g") for ev in pick]
                rest = [ev for ev in sections[k] if ev not in pick]
                sections[k] = rest
                sections[k - 1] = hoisted + sections[k - 1]
            new_schedules.append([ev for sec in sections for ev in sec])
        schedules = new_schedules

    for s in range(S):
        want_seg(s)
    for s in range(S):
        want_x0g(s)

    assert next_shared[0] == ns, (next_shared, ns)
    n_slots = base_uniq + max(next_uniq[0], next_uniq[1])
    slot_maps = []
    for gi in (0, 1):
        m = dict(slot_of)
        for r, v in uniq_slot[gi].items():
            m[r] = base_uniq + v
        slot_maps.append(m)

    return dict(
        so_groups=part,
        schedules=schedules,
        slot_maps=slot_maps,
        n_slots=n_slots,
        seg_first_use=seg_first_use,
        x0g_order=x0g_order,
        paths=paths,
    )


N_TMP = 30  # rotating path-temporary slots in the arena
OUT_RING = 4  # output staging ring (segments)
X1_CHUNKS = (1, 1, 2, 4, 4, 4)  # x1 DMA chunk sizes, first-use order
SLAB_SIZES = (8, 16) + (24,) * 12  # coeff-diagonal matrices per DMA slab


def _build_bass(plan, dtype_name, warmup):
    import concourse.bacc as bacc
    import concourse.mybir as mybir
    from concourse.ap import AP
    from concourse.tile import TileContext

    dt = mybir.dt.float32 if dtype_name == F32 else mybir.dt.bfloat16
    MULT = mybir.AluOpType.mult

    nc = bacc.Bacc("TRN2", debug=False)

    schedules = plan["schedules"]
    slot_maps = plan["slot_maps"]
    n_slots = plan["n_slots"]
    seg_first_use = plan["seg_first_use"]
    x0g_order = plan["x0g_order"]

    n_mms = sum(
        len(ev[1]) for sched in schedules for ev in sched if ev[0] == "path"
    )
    slab_sizes = []
    rem = n_mms
    for sz in SLAB_SIZES:
        if rem <= 0:
            break
        slab_sizes.append(min(sz, rem))
        rem -= slab_sizes[-1]
    while rem > 0:
        slab_sizes.append(min(24, rem))
        rem -= slab_sizes[-1]
    slab_off = [0]
    for sz in slab_sizes:
        slab_off.append(slab_off[-1] + sz)
    max_slab = max(slab_sizes)

    pos_of_seg = {s: i for i, s in enumerate(seg_first_use)}

    x1t_d = nc.dram_tensor("x1t", [S * U, ZS], dt, kind="ExternalInput")
    x0_d = nc.dram_tensor("x0w", [NELEM, S * U], dt, kind="ExternalInput")
    oh_d = nc.dram_tensor("oh", [NELEM, ZS], dt, kind="ExternalInput")
    cd_d = nc.dram_tensor("cdiag", [U, slab_off[-1] * U], dt, kind="ExternalInput")
    out_d = nc.dram_tensor("outt", [S * U, ZS], dt, kind="ExternalOutput")

    POS_X1 = 0
    POS_X0G = S
    POS_PROD = 2 * S
    POS_TMP = POS_PROD + n_slots
    N_ARENA = POS_TMP + N_TMP

    coeff_order = []

    with TileContext(nc) as tc:
        with tc.tile_pool(name="persist", bufs=1) as persist, tc.tile_pool(
            name="slab", bufs=2
        ) as slab_pool:
            arena = persist.tile([U, N_ARENA * ZS], dt, tag="arena")
            out_sb = persist.tile([U, OUT_RING * ZS], dt, tag="out")
            x0_sb = persist.tile([NELEM, S * U], dt, tag="x0w")
            oh_sb = persist.tile([NELEM, ZS], dt, tag="oh")
            wm_w = persist.tile([U, 32], dt, tag="wmw")
            wm_r = persist.tile([U, 256], dt, tag="wmr")

            pstep = [int(arena[:, 0:ZS].ap[0][0]), U]

            def seg(pos, n=1):
                return arena[:, pos * ZS : (pos + n) * ZS]

            def ref_pos(r, smap):
                kind, key = r
                if kind == "x1":
                    return POS_X1 + pos_of_seg[key]
                if kind == "x0g":
                    return POS_X0G + key
                return POS_PROD + smap[r]

            def group_ap(pos_list):
                k = len(pos_list)
                if k == 1:
                    return seg(pos_list[0])
                d = pos_list[1] - pos_list[0]
                assert abs(d) <= 63, pos_list
                if k == 3:
                    assert pos_list[2] - pos_list[1] == d, pos_list
                a = seg(pos_list[0])
                return AP(a.tensor, a.offset, [pstep, [d * ZS, k], [1, ZS]])

            if warmup > 0:
                nc.gpsimd.memset(wm_w[:], 0.0)
                nc.gpsimd.memset(wm_r[:], 0.0)

            # DMA layout: the scalar-triggered queue carries the small
            # gather inputs (done before the DVE stream starts) and later
            # the per-segment outputs; the sync queue carries x1 chunks
            # and the bulk coeff-diagonal slabs, serialized so only one
            # queue streams during the DVE-bound phase (concurrent bulk
            # DMA costs ~20% DVE throughput).
            nc.scalar.dma_start(out=x0_sb[:], in_=x0_d[:])
            nc.scalar.dma_start(out=oh_sb[:], in_=oh_d[:])
            ci = 0
            slabs_started = {}

            def start_slab(si, st):
                # plain 2D copy (host pre-transposed cdiag): 128 descriptors
                nc.sync.dma_start(
                    out=st[:, : slab_sizes[si] * U],
                    in_=cd_d[:, slab_off[si] * U : slab_off[si + 1] * U],
                )

            for ki, csz in enumerate(X1_CHUNKS):
                nc.sync.dma_start(
                    out=seg(POS_X1 + ci, csz).rearrange("p (g z) -> p g z", g=csz),
                    in_=x1t_d[ci * U : (ci + csz) * U, :].rearrange(
                        "(g p) z -> p g z", p=U
                    ),
                )
                ci += csz
                if ki == 2:
                    st = slab_pool.tile([U, max_slab * U], dt, tag="slab", name="slab0")
                    slabs_started[0] = st
                    start_slab(0, st)
                if ki == 3:
                    st = slab_pool.tile([U, max_slab * U], dt, tag="slab", name="slab1")
                    slabs_started[1] = st
                    start_slab(1, st)

            # PE warmup + gathers
            with tc.tile_pool(name="gpsum", bufs=4, space="PSUM") as gpsum:
                if warmup > 0:
                    wt = gpsum.tile([32, 256], mybir.dt.float32, tag="warm", bufs=1)
                    for i in range(warmup):
                        nc.tensor.matmul(
                            wt[:],
                            wm_w[:, 0:32],
                            wm_r[:],
                            start=(i == 0),
                            stop=(i == warmup - 1),
                        )
                    nc.scalar.copy(out=wm_r[:32, 0:8], in_=wt[:, 0:8])
                for s0 in x0g_order:
                    pt = gpsum.tile([U, ZS], mybir.dt.float32, tag="gps")
                    nc.tensor.matmul(
                        pt[:],
                        x0_sb[:, s0 * U : (s0 + 1) * U],
                        oh_sb[:],
                        start=True,
                        stop=True,
                    )
                    nc.scalar.copy(out=seg(POS_X0G + s0), in_=pt[:])

            tmp_next = [0]

            def alloc_tmp(n):
                base = tmp_next[0] % N_TMP
                if base + n > N_TMP:
                    tmp_next[0] += N_TMP - base
                    base = 0
                tmp_next[0] += n
                return POS_TMP + base

            slab_state = {"idx": -1, "tile": None}

            def weight_for(c):
                gidx = len(coeff_order)
                coeff_order.append(c)
                si = 0
                while slab_off[si + 1] <= gidx:
                    si += 1
                if slab_state["idx"] != si:
                    slab_state["idx"] = si
                    if si in slabs_started:
                        st = slabs_started[si]
                    else:
                        st = slab_pool.tile(
                            [U, max_slab * U], dt, tag="slab", name=f"slab{si}"
                        )
                        start_slab(si, st)
                    slab_state["tile"] = st
                st = slab_state["tile"]
                sk = gidx - slab_off[si]
                return st[:, sk * U : (sk + 1) * U]

            so_count = [0]

            for gi, sched in enumerate(schedules):
                smap = slot_maps[gi]
                sos = plan["so_groups"][gi]
                mm_idx = 0
                first_mm = {}
                last_mm = {}
                for ev in sched:
                    if ev[0] != "path":
                        continue
                    for d, r1, r2, c, so in ev[1]:
                        if so not in first_mm:
                            first_mm[so] = mm_idx
                        last_mm[so] = mm_idx
                        mm_idx += 1

                acc = {}
                with tc.tile_pool(
                    name=f"acc{gi}", bufs=8, space="PSUM"
                ) as acc_pool:
                    mm_idx = 0
                    for ev in sched:
                        kind = ev[0]
                        if kind == "build":
                            atoms = ev[1]
                            eng = nc.gpsimd if len(ev) > 2 and ev[2] == "g" else nc.vector
                            outs = [POS_PROD + smap[a[0]] for a in atoms]
                            i0s = [ref_pos(a[1], smap) for a in atoms]
                            i1s = [ref_pos(a[2], smap) for a in atoms]
                            eng.tensor_tensor(
                                out=group_ap(outs),
                                in0=group_ap(i0s),
                                in1=group_ap(i1s),
                                op=MULT,
                            )
                        elif kind == "path":
                            pinfos = ev[1]
                            if pinfos[0][2] is not None:
                                k = len(pinfos)
                                t0 = alloc_tmp(k)
                                nc.vector.tensor_tensor(
                                    out=group_ap(list(range(t0, t0 + k))),
                                    in0=group_ap(
                                        [ref_pos(p[1], smap) for p in pinfos]
                                    ),
                                    in1=group_ap(
                                        [ref_pos(p[2], smap) for p in pinfos]
                                    ),
                                    op=MULT,
                                )
                                rhs_pos = list(range(t0, t0 + k))
                            else:
                                rhs_pos = [ref_pos(pinfos[0][1], smap)]
                            for (d, r1, r2, c, so), rp in zip(pinfos, rhs_pos):
                                if so not in acc:
                                    acc[so] = acc_pool.tile(
                                        [U, ZS],
                                        mybir.dt.float32,
                                        tag=f"acc{sos.index(so)}",
                                        name=f"acc_{so}",
                                        bufs=1,
                                    )
                                blk = weight_for(c)
                                nc.tensor.matmul(
                                    acc[so][:],
                                    blk,
                                    seg(rp),
                                    start=(mm_idx == first_mm[so]),
                                    stop=(mm_idx == last_mm[so]),
                                )
                                mm_idx += 1
                        else:  # so_done
                            so = ev[1]
                            ring = so_count[0] % OUT_RING
                            so_count[0] += 1
                            if so in acc:
                                nc.scalar.copy(
                                    out=out_sb[:, ring * ZS : (ring + 1) * ZS],
                                    in_=acc[so][:],
                                )
                            else:
                                nc.vector.memset(
                                    out_sb[:, ring * ZS : (ring + 1) * ZS], 0.0
                                )
                            nc.scalar.dma_start(
                                out=out_d[so * U : (so + 1) * U, :],
                                in_=out_sb[:, ring * ZS : (ring + 1) * ZS],
                            )
                    done = set(
                        ev[1] for ev in sched if ev[0] == "so_done"
                    )
                    for so in sos:
                        if so not in done:
                            ring = so_count[0] % OUT_RING
                            so_count[0] += 1
                            nc.vector.memset(
                                out_sb[:, ring * ZS : (ring + 1) * ZS], 0.0
                            )
                            nc.scalar.dma_start(
                                out=out_d[so * U : (so + 1) * U, :],
                                in_=out_sb[:, ring * ZS : (ring + 1) * ZS],
                            )

    nc.compile()
    return nc, coeff_order


def _ensure_ntff_hook():
    """Register the axon NTFF profiling hook if the image's antenv lacks
    axon_hooks (bass_utils needs it for trace=True under axon)."""
    import sys
    import types

    try:
        import antenv.axon_hooks  # noqa: F401

        return
    except ImportError:
        pass
    import contextlib
    import ctypes

    so_path = "/opt/axon/libaxon_pjrt.so"
    if not os.path.exists(so_path):
        return
    lib = ctypes.CDLL(so_path)
    if not hasattr(lib, "axon_start_nrt_profile"):
        return
    lib.axon_start_nrt_profile.argtypes = [
        ctypes.POINTER(ctypes.c_int64),
        ctypes.c_size_t,
    ]
    lib.axon_start_nrt_profile.restype = ctypes.c_int64
    lib.axon_stop_nrt_profile.argtypes = [ctypes.c_char_p]
    lib.axon_stop_nrt_profile.restype = ctypes.c_int64

    @contextlib.contextmanager
    def _hook(output_dir, device_ids):
        import jax

        jax.devices()
        if device_ids:
            ids = (ctypes.c_int64 * len(device_ids))(*device_ids)
            rc = lib.axon_start_nrt_profile(ids, len(device_ids))
        else:
            rc = lib.axon_start_nrt_profile(None, 0)
        if rc != 0:
            raise RuntimeError(f"axon_start_nrt_profile rc={rc}")
        try:
            yield
        finally:
            n = lib.axon_stop_nrt_profile(str(output_dir).encode())
            print(f"profile: {n} file(s) written to {output_dir}")

    mod = types.ModuleType("antenv.axon_hooks")
    state = {"hook": _hook}
    mod.get_axon_ntff_profile_hook = lambda: state["hook"]
    mod.set_axon_ntff_profile_hook = lambda h: state.__setitem__("hook", h)
    import antenv

    antenv.axon_hooks = mod
    sys.modules["antenv.axon_hooks"] = mod


def kernel(x0, x1, coeff1, coeff2, coeff3, i0, idx1, idx2, idx3):
    global LAST_EXEC_NS, LAST_RESULTS
    from concourse.bass_utils import run_bass_kernel_spmd

    x0 = np.asarray(x0, dtype=np.float32)
    x1 = np.asarray(x1, dtype=np.float32)
    i0 = np.asarray(i0).astype(np.int64)
    idxs = [np.asarray(a) for a in (idx1, idx2, idx3)]
    coeffs = [np.asarray(c, dtype=np.float32) for c in (coeff1, coeff2, coeff3)]

    dtype_name = os.environ.get("KERNEL_DTYPE", "bfloat16")
    warmup = int(os.environ.get("KERNEL_WARMUP", "12"))
    npdt = np.float32
    if dtype_name != F32:
        import ml_dtypes

        npdt = ml_dtypes.bfloat16

    plan = _build_plan(idxs, coeffs)
    nc, coeff_order = _build_bass(plan, dtype_name, warmup)

    # replicate the device-side slab trimming exactly
    n_co = len(coeff_order)
    total = 0
    rem = n_co
    for sz in SLAB_SIZES:
        if rem <= 0:
            break
        take = min(sz, rem)
        total += take
        rem -= take
    while rem > 0:
        take = min(24, rem)
        total += take
        rem -= take
    cdiag = np.zeros((U, total * U), dtype=npdt)
    uu = np.arange(U)
    for gidx, c in enumerate(coeff_order):
        cdiag[uu, gidx * U + uu] = npdt(c)

    # host-side layout: x1 transposed per core with segments permuted into
    # first-use order
    perm = plan["seg_first_use"]
    in_maps = []
    eye = np.arange(NELEM)
    x0c = x0.astype(npdt)
    for c in range(NCORES):
        zl, zh = c * ZS, (c + 1) * ZS
        shard = x1[zl:zh].reshape(ZS, S, U)
        x1t = np.ascontiguousarray(
            shard[:, perm, :].transpose(1, 2, 0).reshape(S * U, ZS)
        ).astype(npdt)
        oh = (i0[zl:zh][None, :] == eye[:, None]).astype(npdt)
        in_maps.append({"x1t": x1t, "x0w": x0c, "oh": oh, "cdiag": cdiag})

    trace = os.environ.get("BASS_TRACE", "") not in ("", "0")
    if trace:
        _ensure_ntff_hook()
    trace_cores = None
    tc_env = os.environ.get("KERNEL_TRACE_CORES", "")
    if tc_env:
        trace_cores = [int(x) for x in tc_env.split(",")]
    res = run_bass_kernel_spmd(
        nc, in_maps, core_ids=list(range(NCORES)), trace=trace,
        trace_cores=trace_cores,
    )
    LAST_EXEC_NS = res.exec_time_ns
    LAST_RESULTS = res

    out = np.empty((Z, S * U), dtype=np.float32)
    for c in range(NCORES):
        outt = np.asarray(res.results[c]["outt"], dtype=np.float32)
        out[c * ZS : (c + 1) * ZS] = (
            outt.reshape(S, U, ZS).transpose(2, 0, 1).reshape(ZS, S * U)
        )
    return out
